# revision 1
# baseline (speedup 1.0000x reference)
"""Causal self-attention Trainium2 kernel.

Full computation: y = softmax_causal((x@Wq)(x@Wk)^T / sqrt(D)) @ (x@Wv) @ Wp
Sharding: head-parallel over 8 cores (H=8 heads, one per core), both batches
on every core (batch 0 on SBUF partitions 0:64, batch 1 on 64:128).
Each core produces a partial output (its head's contribution to y @ W_proj);
the host sums the 8 partials.
"""

import sys

sys.path.insert(0, "/opt/trn_rl_repo")

from contextlib import ExitStack

import numpy as np

import concourse.bass as bass
import concourse.mybir as mybir
import concourse.tile as tile
from concourse import bacc

B, T, C, H, D = 2, 4096, 512, 8, 64
BT = B * T  # 8192
NCORES = 8
NC_CH = C // 128  # 4 contraction chunks for the QKV projection
NQT = T // 512  # 8 q-tiles per batch
NKT = T // 128  # 32 k-tiles per batch
KGRP = 3  # k-tiles per exp group (3 PSUM banks, double buffered)

f32 = mybir.dt.float32
f32r = mybir.dt.float32r
bf16 = mybir.dt.bfloat16


def _r(ap):
    return ap  # tiles are fp32r-typed now


def build_kernel() -> bass.Bass:
    nc = bacc.Bacc()

    xT = nc.dram_tensor("xT", [C, BT], bf16, kind="ExternalInput")
    wq = nc.dram_tensor("wq", [C, D], bf16, kind="ExternalInput")
    wk = nc.dram_tensor("wk", [C, D], bf16, kind="ExternalInput")
    wv = nc.dram_tensor("wv", [C, D], bf16, kind="ExternalInput")
    # wp row D is zeros; rows 0:D are this head's W_proj slice.
    wp = nc.dram_tensor("wp", [D + 1, C], f32r, kind="ExternalInput")
    ev = nc.dram_tensor("ev", [D + 1, 2], f32r, kind="ExternalInput")
    ones64 = nc.dram_tensor("ones64", [64], f32r, kind="ExternalInput")
    outp = nc.dram_tensor("outp", [BT, C], f32, kind="ExternalOutput")

    xTr = xT[:, :].rearrange("(a p) t -> a p t", p=128)  # [4, 128, BT]

    with tile.TileContext(nc) as tc, ExitStack() as ctx:
        singles = ctx.enter_context(tc.tile_pool(name="singles", bufs=1))

        # Persistent SBUF tensors
        qT = singles.tile([128, T], f32r)  # [0:64]=batch0 head dims, [64:128]=batch1
        kT = singles.tile([128, T], f32r)
        v_sb = singles.tile([128, B * NKT, D + 1], f32r)  # v tiles + ones column
        yT = singles.tile([D + 1, BT], f32r)  # unnormalized y^T; row D = softmax sums
        wq_sb = singles.tile([128, NC_CH, D], bf16)
        wk_sb = singles.tile([128, NC_CH, D], bf16)
        wv_sb = singles.tile([128, NC_CH, D], bf16)
        wp_sb = singles.tile([D + 1, C], f32r)
        e_sb = singles.tile([D + 1, 2], f32r)

        nc.sync.dma_start(wq_sb[:], wq[:, :].rearrange("(a p) d -> p a d", p=128))
        nc.sync.dma_start(wk_sb[:], wk[:, :].rearrange("(a p) d -> p a d", p=128))
        nc.sync.dma_start(wv_sb[:], wv[:, :].rearrange("(a p) d -> p a d", p=128))
        nc.sync.dma_start(wp_sb[:], wp[:, :])
        nc.sync.dma_start(e_sb[:], ev[:, :])
        o = ones64[:]
        ones_bcast = bass.AP(tensor=o.tensor, offset=o.offset, ap=[[0, 128], [1, 64]])
        nc.gpsimd.dma_start(out=v_sb[:, :, D], in_=ones_bcast)

        # ---------------- Phase 1: QKV projection ----------------
        with (
            tc.tile_pool(name="p1x", bufs=4) as xpool,
            tc.tile_pool(name="p1qk", bufs=2, space="PSUM") as psqk,
            tc.tile_pool(name="p1v", bufs=4, space="PSUM") as psv,
        ):
            for j in range(NQT):
                for b in range(B):
                    t0 = b * T + j * 512
                    xt = xpool.tile([128, NC_CH, 512], bf16, tag="xt")
                    for c in range(NC_CH):
                        nc.sync.dma_start(xt[:, c, :], xTr[c, :, t0 : t0 + 512])
                    lo, hi = 64 * b, 64 * b + 64
                    tp = (0, 64) if b == 1 else None
                    pq = psqk.tile([128, 512], f32, tag="pq")
                    pk = psqk.tile([128, 512], f32, tag="pk")
                    for c in range(NC_CH):
                        nc.tensor.matmul(
                            pq[lo:hi, :],
                            lhsT=_r(wq_sb[:, c, :]),
                            rhs=_r(xt[:, c, :]),
                            start=(c == 0),
                            stop=(c == NC_CH - 1),
                            tile_position=tp,
                        )
                    for c in range(NC_CH):
                        nc.tensor.matmul(
                            pk[lo:hi, :],
                            lhsT=_r(wk_sb[:, c, :]),
                            rhs=_r(xt[:, c, :]),
                            start=(c == 0),
                            stop=(c == NC_CH - 1),
                            tile_position=tp,
                        )
                    nc.vector.tensor_copy(
                        out=qT[lo:hi, j * 512 : (j + 1) * 512], in_=pq[lo:hi, :]
                    )
                    nc.vector.tensor_copy(
                        out=kT[lo:hi, j * 512 : (j + 1) * 512], in_=pk[lo:hi, :]
                    )
                    # v in natural [T, D] layout: x-tile chunks as stationary operand
                    for rr in range(4):
                        pv = psv.tile([128, D], f32, tag="pv")
                        for c in range(NC_CH):
                            nc.tensor.matmul(
                                pv[:],
                                lhsT=_r(xt[:, c, rr * 128 : (rr + 1) * 128]),
                                rhs=_r(wv_sb[:, c, :]),
                                start=(c == 0),
                                stop=(c == NC_CH - 1),
                            )
                        rt = b * NKT + j * 4 + rr
                        nc.vector.tensor_copy(out=v_sb[:, rt, 0:D], in_=pv[:])

        # ---------------- Phase 2: causal attention ----------------
        with (
            tc.tile_pool(name="p2p", bufs=3) as ppool,
            tc.tile_pool(name="p2s", bufs=2, space="PSUM") as pss,
            tc.tile_pool(name="p2y", bufs=2, space="PSUM") as psy,
        ):
            for j in range(NQT):
                q0 = j * 512
                nkt = 4 * (j + 1)  # causal k-tiles for this q block
                groups = [
                    list(range(s, min(s + KGRP, nkt))) for s in range(0, nkt, KGRP)
                ]
                yps = [
                    psy.tile([D + 1, 512], f32, tag="y", name=f"y_{j}_{b}")
                    for b in range(B)
                ]
                for g in groups:
                    for b in range(B):
                        lo, hi = 64 * b, 64 * b + 64
                        s4 = pss.tile([128, KGRP, 512], f32, tag="s")
                        for ui, kt in enumerate(g):
                            nc.tensor.matmul(
                                s4[:, ui, :],
                                lhsT=_r(kT[lo:hi, kt * 128 : (kt + 1) * 128]),
                                rhs=_r(qT[lo:hi, q0 : q0 + 512]),
                                start=True,
                                stop=True,
                            )
                        nu = len(g)
                        p4 = ppool.tile([128, KGRP, 512], f32r, tag="p")
                        # exp(s/sqrt(D)); scores are O(1) so no max subtraction
                        nc.scalar.activation(
                            out=p4[:, 0:nu, :],
                            in_=s4[:, 0:nu, :],
                            func=mybir.ActivationFunctionType.Exp,
                            scale=0.125,
                        )
                        for ui, kt in enumerate(g):
                            dlt = kt * 128 - q0
                            if dlt > -128:
                                # keep where (q0+col) >= (kt*128+p)
                                nc.gpsimd.affine_select(
                                    out=p4[:, ui, :],
                                    in_=p4[:, ui, :],
                                    compare_op=mybir.AluOpType.is_ge,
                                    fill=0.0,
                                    base=-dlt,
                                    channel_multiplier=-1,
                                    pattern=[[1, 512]],
                                )
                        for ui, kt in enumerate(g):
                            nc.tensor.matmul(
                                yps[b][:],
                                lhsT=_r(v_sb[:, b * NKT + kt, :]),
                                rhs=_r(p4[:, ui, :]),
                                start=(kt == 0),
                                stop=(kt == nkt - 1),
                            )
                for b in range(B):
                    nc.vector.tensor_copy(
                        out=yT[:, b * T + q0 : b * T + q0 + 512], in_=yps[b][:]
                    )

        # ---------------- Phase 3: c_proj partial + normalization ----------------
        with (
            tc.tile_pool(name="p3o", bufs=3) as opool,
            tc.tile_pool(name="p3ps", bufs=2, space="PSUM") as pso,
        ):
            for r in range(BT // 128):
                lhsT = yT[:, r * 128 : (r + 1) * 128]  # [65, 128]
                po = pso.tile([128, C], f32, tag="po")
                pu = pso.tile([128, 2], f32, tag="pu")
                nc.tensor.matmul(po[:], lhsT=_r(lhsT), rhs=_r(wp_sb[:]), start=True, stop=True)
                nc.tensor.matmul(pu[:], lhsT=_r(lhsT), rhs=_r(e_sb[:]), start=True, stop=True)
                recip = opool.tile([128, 1], f32, tag="recip")
                nc.vector.reciprocal(recip[:], pu[:, 0:1])
                ot = opool.tile([128, C], f32, tag="ot")
                nc.vector.tensor_scalar_mul(ot[:], in0=po[:], scalar1=recip[:])
                nc.sync.dma_start(outp[r * 128 : (r + 1) * 128, :], ot[:])

    nc.compile()
    return nc


_cache: dict = {}


def _get_nc() -> bass.Bass:
    if "nc" not in _cache:
        _cache["nc"] = build_kernel()
    return _cache["nc"]


def make_in_maps(x, W_attn, W_proj):
    import ml_dtypes
    xTq = np.ascontiguousarray(x.reshape(BT, C).T).astype(ml_dtypes.bfloat16)
    in_maps = []
    for i in range(NCORES):
        wp_pad = np.zeros((D + 1, C), dtype=np.float32)
        wp_pad[:D] = W_proj[i * D : (i + 1) * D, :]
        ev = np.zeros((D + 1, 2), dtype=np.float32)
        ev[D, 0] = 1.0
        in_maps.append(
            {
                "xT": xTq,
                "ev": ev,
                "ones64": np.ones(64, dtype=np.float32),
                "wq": np.ascontiguousarray(W_attn[:, i * D : (i + 1) * D]).astype(ml_dtypes.bfloat16),
                "wk": np.ascontiguousarray(W_attn[:, C + i * D : C + (i + 1) * D]).astype(ml_dtypes.bfloat16),
                "wv": np.ascontiguousarray(
                    W_attn[:, 2 * C + i * D : 2 * C + (i + 1) * D]
                ).astype(ml_dtypes.bfloat16),
                "wp": wp_pad,
            }
        )
    return in_maps


def kernel(x, W_attn, W_proj, _trace=False):
    from concourse.bass_utils import run_bass_kernel_spmd

    nc = _get_nc()
    in_maps = make_in_maps(
        np.asarray(x, dtype=np.float32),
        np.asarray(W_attn, dtype=np.float32),
        np.asarray(W_proj, dtype=np.float32),
    )
    res = run_bass_kernel_spmd(
        nc, in_maps, core_ids=list(range(NCORES)), trace=_trace
    )
    out = np.zeros((BT, C), dtype=np.float32)
    for r in res.results:
        out += r["outp"]
    out = out.reshape(B, T, C)
    if _trace:
        return out, res
    return out

